# revision 1
# baseline (speedup 1.0000x reference)
"""Trainium2 Bass kernel for nn_DecoderCrossMSA (Swin-style shifted-window
cross-attention).

Strategy: data-parallel over batch (8 batches -> 8 cores). Host prepares, per
core, feature-major window-ordered activations (token axis permuted so every
8x8 shifted window is a contiguous 64-token run; roll folded into the
permutation). Device does:
  - 4 input projections in bf16 (Q scaled by 1/sqrt(32), biases folded where
    possible),
  - windowed attention: S^T = K^T.T @ Q^T per (window, head) on the tensor
    engine, softmax as exp (scalar engine) x static exp-bias table (relative
    position bias + shift masks, multiplicative so masking is exact zeros),
    row-sums via ones-matmul, normalization folded into P,
  - AV matmuls emit feature-major attention output directly,
  - 2 output projections (+ biases folded with V biases) in bf16, fp32 out.
Host inverse-permutes/transposes and reassembles the full outputs.
"""

import os

import numpy as np
import ml_dtypes

EMB = 512
HEADS = 16
WS = 8
B = 8
HW = 64
N = HW * HW
EH = EMB // HEADS          # 32
WN = HW // WS              # 8
SHIFT = WS // 2            # 4
NW = WN * WN               # 64 windows
WT = WS * WS               # 64 tokens per window
NCORES = 8
NBLK = 8                   # token blocks per core (512 tokens each)
BLKT = N // NBLK           # 512
NPAIR = 32                 # window pairs per core
MASK_NEG = -30000.0

_bf16 = ml_dtypes.bfloat16


def _build_perm(shift):
    """perm[t] = token index n for window-ordered position t.

    t = ((i*WN + j) * WT) + (w1*WS + w2); grid row = (WS*i + w1 + shift) mod
    HW, col = (WS*j + w2 + shift) mod HW. Inputs are read through the rolled
    grid (shift=SHIFT); outputs are written back WITHOUT inverting the roll
    (shift=0) — the reference's _unwindow does not undo the roll.
    """
    i, j, w1, w2 = np.meshgrid(
        np.arange(WN), np.arange(WN), np.arange(WS), np.arange(WS), indexing="ij"
    )
    r = (WS * i + w1 + shift) % HW
    c = (WS * j + w2 + shift) % HW
    return (r * HW + c).reshape(-1)


_PERM = _build_perm(SHIFT)
_OPERM = _build_perm(0)

# Reference splits EMB as (e H): head h lives on strided channels e*HEADS+h.
# Permute projection out-channels so head h is the contiguous block h*EH..:
# new channel h*EH+e = old channel e*HEADS+h.
_RHO = np.array([e * HEADS + h for h in range(HEADS) for e in range(EH)])


def _pair_tables(pos_emb):
    """4 pair-type tables [128, 16*64] bf16 of exp(T)^T, head-replicated.

    T[q, k] = pos_bias[q, k] (+ row mask if window-row i == WN-1)
                         (+ col mask if window-col j == WN-1).
    Table rows = k (2 windows stacked: first window of pair rows 0:64, second
    rows 64:128), free = (16 heads replicated, 64 q).
    pair p = windows (2p, 2p+1): second window is col-masked iff p % 4 == 3;
    both windows row-masked iff p // 4 == WN - 1.
    """
    idx = np.array([[x, y] for x in range(WS) for y in range(WS)])
    rel = idx[None, :, :] - idx[:, None, :] + WS - 1
    bias = pos_emb[rel[:, :, 0], rel[:, :, 1]].astype(np.float64)

    m = np.zeros((WT, WT), dtype=np.float64)
    s = WS * (WS // 2)
    m[-s:, :-s] = MASK_NEG
    m[:-s, -s:] = MASK_NEG
    r = WT // WS
    col = m.reshape(r, WS, r, WS).transpose(1, 0, 3, 2).reshape(WT, WT)

    t0 = bias
    t1 = bias + m          # row-masked  (i == 7)
    t2 = bias + col        # col-masked  (j == 7)
    t3 = bias + m + col    # corner

    def pair_tab(ta, tb):
        # exp, transpose to [k, q], stack windows, replicate over heads
        ea = np.exp(ta).T    # [k, q]
        eb = np.exp(tb).T
        stk = np.concatenate([ea, eb], axis=0)           # [128, 64]
        rep = np.tile(stk, (1, HEADS))                    # [128, 16*64]
        return rep.astype(_bf16)

    # pair types: (normal,normal), (normal,colmask), (rowmask,rowmask),
    # (rowmask,corner)
    return np.stack([
        pair_tab(t0, t0),
        pair_tab(t0, t2),
        pair_tab(t1, t1),
        pair_tab(t1, t3),
    ])


def _pair_type(p):
    row = (p // 4) == WN - 1      # window-row i == 7
    colm = (p % 4) == 3           # second window j == 7
    return (2 if row else 0) + (1 if colm else 0)


def _build_bass(debug=False, stage=99, reps=1):
    import concourse.mybir as mybir
    from concourse import bacc
    from concourse.tile import TileContext

    fp32 = mybir.dt.float32
    bf16 = mybir.dt.bfloat16
    AF = mybir.ActivationFunctionType
    ALU = mybir.AluOpType

    nc = bacc.Bacc()

    # ---- DRAM parameters (per-core) ----
    d_in = {}
    for name in ("cw", "sw", "scw", "shw"):
        d_in[name] = nc.declare_dram_parameter(name, [EMB, N], bf16, isOutput=False)
    for name in ("w1t", "w2t", "wsct", "wsht", "wsot", "wshot"):
        d_in[name] = nc.declare_dram_parameter(name, [EMB, EMB], bf16, isOutput=False)
    for name in ("b1r", "b2r", "bsor", "bshor"):
        d_in[name] = nc.declare_dram_parameter(name, [128, 4], fp32, isOutput=False)
    d_in["ptab"] = nc.declare_dram_parameter(
        "ptab", [4, 128, HEADS * WT], bf16, isOutput=False
    )
    d_in["onesc"] = nc.declare_dram_parameter("onesc", [128, WT], bf16, isOutput=False)
    yso = nc.declare_dram_parameter("yso", [EMB, N], fp32, isOutput=True)
    ysho = nc.declare_dram_parameter("ysho", [EMB, N], fp32, isOutput=True)
    dbg = {}
    if debug:
        for name, shape in (
            ("dbg_ct", [EMB, BLKT]), ("dbg_st", [128, HEADS * WT]),
            ("dbg_pa", [128, HEADS * WT]), ("dbg_pn", [128, HEADS * WT]),
            ("dbg_av", [128, 1024]), ("dbg_v", [128, EMB]),
        ):
            dbg[name] = nc.declare_dram_parameter(name, shape, fp32, isOutput=True)

    with TileContext(nc) as tc:
        with (
            tc.tile_pool(name="const", bufs=1) as cpool,
            tc.tile_pool(name="xg", bufs=2) as xgpool,
            tc.tile_pool(name="cs", bufs=2) as cspool,
            tc.tile_pool(name="v", bufs=5) as vpool,
            tc.tile_pool(name="p", bufs=4) as ppool,
            tc.tile_pool(name="o", bufs=3) as opool,
            tc.tile_pool(name="y", bufs=4) as ypool,
            tc.tile_pool(name="stps", bufs=1, space="PSUM") as stps,
            tc.tile_pool(name="bigps", bufs=2, space="PSUM") as bigps,
        ):
            # ---- constants into SBUF ----
            wts = {}
            for name in ("w1t", "w2t", "wsct", "wsht", "wsot", "wshot"):
                wts[name] = []
                for k in range(4):
                    t = cpool.tile([128, EMB], bf16, tag=f"{name}_{k}")
                    nc.sync.dma_start(t[:], d_in[name][k * 128:(k + 1) * 128, :])
                    wts[name].append(t)
            bias_t = {}
            for name in ("b1r", "b2r", "bsor", "bshor"):
                t = cpool.tile([128, 4], fp32, tag=name)
                nc.sync.dma_start(t[:], d_in[name][:])
                bias_t[name] = t
            ptab_t = []
            for i in range(4):
                t = cpool.tile([128, HEADS * WT], bf16, tag=f"ptab{i}")
                nc.sync.dma_start(t[:], d_in["ptab"][i])
                ptab_t.append(t)
            ones_t = cpool.tile([128, WT], bf16, tag="onesc")
            nc.sync.dma_start(ones_t[:], d_in["onesc"][:])

            for blk0 in range(NBLK * reps):
                blk = blk0 % NBLK
                c0 = blk * BLKT
                # ---- stage inputs [e_in chunk, 512 tokens] ----
                xg = {}
                for tname in ("cw", "sw", "scw", "shw"):
                    xg[tname] = []
                    for k in range(4):
                        t = xgpool.tile([128, BLKT], bf16, tag=f"xg_{tname}_{k}")
                        nc.sync.dma_start(
                            t[:], d_in[tname][k * 128:(k + 1) * 128, c0:c0 + BLKT]
                        )
                        xg[tname].append(t)

                # ---- Q/K projections (feature-major) ----
                cs = {}
                for tname, wname, bname in (
                    ("cw", "w1t", "b1r"), ("sw", "w2t", "b2r")
                ):
                    cs[tname] = []
                    for m in range(4):
                        ps = bigps.tile([128, BLKT], fp32, tag="big", name="pspj")
                        for k in range(4):
                            nc.tensor.matmul(
                                ps[:],
                                lhsT=wts[wname][k][:, m * 128:(m + 1) * 128],
                                rhs=xg[tname][k][:],
                                start=(k == 0),
                                stop=(k == 3),
                            )
                        out = cspool.tile([128, BLKT], bf16, tag=f"cs_{tname}_{m}")
                        nc.scalar.activation(
                            out[:], ps[:], AF.Identity,
                            bias=bias_t[bname][:, m:m + 1],
                        )
                        cs[tname].append(out)
                cT, sT = cs["cw"], cs["sw"]
                if debug and blk == 0:
                    for m in range(4):
                        nc.gpsimd.dma_start(
                            dbg["dbg_ct"][m * 128:(m + 1) * 128, :], cT[m][:]
                        )

                # ---- V projections (token-major), per pair ----
                vsc_l, vsh_l = [], []
                for p in range(4):
                    t0 = p * 128
                    for tname, wname, dst in (
                        ("scw", "wsct", vsc_l), ("shw", "wsht", vsh_l)
                    ):
                        ps = bigps.tile([128, EMB], fp32, tag="big", name="psv")
                        for k in range(4):
                            nc.tensor.matmul(
                                ps[:],
                                lhsT=xg[tname][k][:, t0:t0 + 128],
                                rhs=wts[wname][k][:],
                                start=(k == 0),
                                stop=(k == 3),
                            )
                        out = vpool.tile([128, EMB], bf16, tag=f"v_{tname}")
                        nc.vector.tensor_copy(out[:], ps[:])
                        dst.append(out)

                if stage <= 1:
                    continue
                # ---- attention per pair ----
                osc = opool.tile([128, 4 * BLKT], bf16, tag="osc")
                osh = opool.tile([128, 4 * BLKT], bf16, tag="osh")
                for p in range(4):
                    pg = blk * 4 + p
                    ptype = _pair_type(pg)
                    t0 = p * 128
                    # S^T psum: 4 banks; head h lands in bank h%4 == its PE
                    # row-group, so concurrent row-tiled matmuls never share a
                    # (bank, partition) pair (HW write-collision otherwise).
                    st = stps.tile([128, 4 * 512], fp32, tag="st")
                    for h in range(HEADS):
                        m, r = h // 4, (h % 4) * 32
                        s0 = (h % 4) * 512 + (h // 4) * WT
                        for wi in range(2):
                            o0 = t0 + wi * WT
                            nc.tensor.matmul(
                                st[wi * WT:(wi + 1) * WT, s0:s0 + WT],
                                lhsT=sT[m][r:r + 32, o0:o0 + WT],
                                rhs=cT[m][r:r + 32, o0:o0 + WT],
                                start=True, stop=True,
                                tile_position=(r, wi * WT),
                            )
                    # compact [128, 4, 4, 64] view of the used st slots
                    st_v = st[:].rearrange(
                        "p (b s q) -> p b s q", b=4, s=8, q=WT
                    )[:, :, 0:4, :]
                    if debug and blk == 0 and p == 0:
                        t = ypool.tile([128, HEADS * WT], fp32, tag="dbg")
                        tv = t[:].rearrange("p (b s q) -> p b s q", b=4, s=4, q=WT)
                        nc.scalar.activation(tv, st_v, AF.Copy)
                        nc.gpsimd.dma_start(dbg["dbg_st"][:], t[:])
                    pe = ppool.tile([128, HEADS * WT], bf16, tag="pe")
                    pe_v = pe[:].rearrange("p (b s q) -> p b s q", b=4, s=4, q=WT)
                    nc.scalar.activation(pe_v, st_v, AF.Exp)
                    pa = ppool.tile([128, HEADS * WT], bf16, tag="pa")
                    nc.vector.tensor_tensor(
                        pa[:], pe[:], ptab_t[ptype][:], ALU.mult
                    )
                    if stage <= 2:
                        continue
                    dd = bigps.tile([128, HEADS * WT], fp32, tag="big", name="dd")
                    for wi in range(2):
                        sl = slice(wi * WT, (wi + 1) * WT)
                        for half in range(2):
                            fs = slice(half * 512, (half + 1) * 512)
                            nc.tensor.matmul(
                                dd[sl, fs],
                                lhsT=ones_t[sl, :],
                                rhs=pa[sl, fs],
                                start=True, stop=True,
                                tile_position=(wi * WT, wi * WT),
                            )
                    rd = ppool.tile([128, HEADS * WT], fp32, tag="rd")
                    nc.vector.reciprocal(rd[:], dd[:])
                    pn = ppool.tile([128, HEADS * WT], bf16, tag="pn")
                    nc.vector.tensor_tensor(pn[:], pa[:], rd[:], ALU.mult)
                    if debug and blk == 0 and p == 0:
                        nc.gpsimd.dma_start(dbg["dbg_pa"][:], pa[:])
                        nc.gpsimd.dma_start(dbg["dbg_pn"][:], pn[:])
                        nc.gpsimd.dma_start(dbg["dbg_v"][:], vsc_l[0][:])
                    if stage <= 3:
                        continue

                    # AV psum: bank = window half == PE row-group of the MM.
                    # free = wi*512 + m*128 + q
                    av_sc = bigps.tile([128, 1024], fp32, tag="big", name="avsc")
                    av_sh = bigps.tile([128, 1024], fp32, tag="big", name="avsh")
                    for h in range(HEADS):
                        m, r = h // 4, (h % 4) * 32
                        ps0 = ((h % 4) * 4 + h // 4) * WT   # pn slot for head h
                        for wi in range(2):
                            sl = slice(wi * WT, (wi + 1) * WT)
                            f0 = wi * 512 + m * WT
                            for vt, av in ((vsc_l[p], av_sc), (vsh_l[p], av_sh)):
                                nc.tensor.matmul(
                                    av[r:r + 32, f0:f0 + WT],
                                    lhsT=vt[sl, h * 32:(h + 1) * 32],
                                    rhs=pn[sl, ps0:ps0 + WT],
                                    start=True, stop=True,
                                    tile_position=(wi * WT, r),
                                )
                    if debug and blk == 0 and p == 0:
                        t = ypool.tile([128, 1024], fp32, tag="dbg2")
                        nc.scalar.activation(t[:], av_sc[:], AF.Copy)
                        nc.gpsimd.dma_start(dbg["dbg_av"][:], t[:])
                    # scatter [128, (2 wi, 4 m, 64 q)] into O tiles
                    for o_t, av, eng in ((osc, av_sc, nc.scalar),
                                         (osh, av_sh, nc.vector)):
                        src = av[:].rearrange("p (w m q) -> p m w q", w=2, m=8,
                                              q=WT)[:, 0:4]
                        dstv = o_t[:].rearrange("p (m t) -> p m t", m=4)
                        dst = dstv[:, :, t0:t0 + 128].rearrange(
                            "p m (w q) -> p m w q", w=2
                        )
                        if eng is nc.scalar:
                            nc.scalar.activation(dst, src, AF.Copy)
                        else:
                            nc.vector.tensor_copy(dst, src)

                if stage <= 4:
                    continue
                # ---- output projections ----
                for o_t, wname, bname, y_h in (
                    (osc, "wsot", "bsor", yso), (osh, "wshot", "bshor", ysho)
                ):
                    for mo in range(4):
                        ps = bigps.tile([128, BLKT], fp32, tag="big", name="pso")
                        for k in range(4):
                            nc.tensor.matmul(
                                ps[:],
                                lhsT=wts[wname][k][:, mo * 128:(mo + 1) * 128],
                                rhs=o_t[:, k * BLKT:(k + 1) * BLKT],
                                start=(k == 0),
                                stop=(k == 3),
                            )
                        y_sb = ypool.tile([128, BLKT], fp32, tag="y")
                        nc.scalar.activation(
                            y_sb[:], ps[:], AF.Identity,
                            bias=bias_t[bname][:, mo:mo + 1],
                        )
                        nc.sync.dma_start(
                            y_h[mo * 128:(mo + 1) * 128, c0:c0 + BLKT], y_sb[:]
                        )
    nc.compile()
    return nc


_NC_CACHE = {}
LAST_RESULT = None


def make_in_maps(content, style, scale, shift, W1, b1, W2, b2, Wsc, bsc,
                 Wsh, bsh, Wso, bso, Wsho, bsho, pos_emb):
    inv = 1.0 / np.sqrt(EMB / HEADS)
    f32 = np.float32

    # head-contiguous channel permutation on projection out-channels (_RHO);
    # inverted on the output-projection in-channels.
    w1t = (np.asarray(W1, f32)[_RHO].T * inv).astype(_bf16)  # [e_in, e_out], scaled
    w2t = np.asarray(W2, f32)[_RHO].T.astype(_bf16)
    wsct = np.asarray(Wsc, f32)[_RHO].T.astype(_bf16)
    wsht = np.asarray(Wsh, f32)[_RHO].T.astype(_bf16)
    # _unwindow emits channels H-major (h*EH+e) == device O-row order, so the
    # output projections are NOT channel-permuted.
    wsot = np.asarray(Wso, f32).T.astype(_bf16)
    wshot = np.asarray(Wsho, f32).T.astype(_bf16)
    b1r = (np.asarray(b1, f32)[_RHO] * inv).reshape(4, 128).T.copy()
    b2r = np.asarray(b2, f32)[_RHO].reshape(4, 128).T.copy()
    # V biases folded into output-projection biases; V channels reach the
    # output projection in unwindow (H-major) order, hence bsc[_RHO].
    bso2 = np.asarray(Wso, f32) @ np.asarray(bsc, f32)[_RHO] + np.asarray(bso, f32)
    bsho2 = (np.asarray(Wsho, f32) @ np.asarray(bsh, f32)[_RHO]
             + np.asarray(bsho, f32))
    bsor = bso2.reshape(4, 128).T.copy()
    bshor = bsho2.reshape(4, 128).T.copy()
    ptab = _pair_tables(np.asarray(pos_emb, f32))
    onesc = np.ones((128, WT), dtype=_bf16)

    common = dict(
        w1t=w1t, w2t=w2t, wsct=wsct, wsht=wsht, wsot=wsot, wshot=wshot,
        b1r=b1r, b2r=b2r, bsor=bsor, bshor=bshor, ptab=ptab, onesc=onesc,
    )
    in_maps = []
    for b in range(NCORES):
        m = dict(common)
        for name, full in (("cw", content), ("sw", style),
                           ("scw", scale), ("shw", shift)):
            x = np.asarray(full[b], f32)[_PERM]           # [N, EMB] window order
            m[name] = np.ascontiguousarray(x.T).astype(_bf16)
        in_maps.append(m)
    return in_maps


def kernel(**inputs):
    global LAST_RESULT
    from concourse.bass_utils import run_bass_kernel_spmd

    in_maps = make_in_maps(**inputs)

    if "nc" not in _NC_CACHE:
        _NC_CACHE["nc"] = _build_bass()
    res = run_bass_kernel_spmd(_NC_CACHE["nc"], in_maps, list(range(NCORES)))
    LAST_RESULT = res

    out_sc = np.empty((B, N, EMB), np.float32)
    out_sh = np.empty((B, N, EMB), np.float32)
    for b in range(NCORES):
        out_sc[b][_OPERM] = res.results[b]["yso"].T
        out_sh[b][_OPERM] = res.results[b]["ysho"].T
    return out_sc, out_sh



# revision 5
# speedup vs baseline: 1.1703x; 1.1703x over previous
"""Trainium2 Bass kernel for nn_DecoderCrossMSA (Swin-style shifted-window
cross-attention).

Strategy: data-parallel over batch (8 batches -> 8 cores). Host prepares, per
core, feature-major window-ordered activations (token axis permuted so every
8x8 shifted window is a contiguous 64-token run; roll folded into the
permutation). Device, per 512-token block (4 window-pairs):
  - Q/K projections feature-major, V projections token-major into a per-pair
    combined tile (sc|sh interleaved per head), all bf16 (Q pre-scaled by
    1/sqrt(32); V biases folded into the output-projection biases),
  - per pair (2 windows, 128 tokens): S^T = K^T.T @ Q^T per head over the
    full 128x128 pair block (cross-window entries junk), exp on the scalar
    engine, multiplied by a static table exp(pos bias + shift masks) with
    zeros on cross-window blocks (masking = exact zeros, head-broadcast via
    stride-0 APs),
  - denominators replicated onto 32-partition groups via ones-lhsT matmuls,
    reciprocal on DVE, one broadcast tensor_tensor per (pair, out) that
    normalizes AND scatters attention output feature-major,
  - AV matmuls with V stationary emit [32e, 128q] per head at the partition
    offset that makes psum columns land feature-major directly,
  - output projections in bf16, bf16 out (host converts to fp32).
Host inverse-permutes/transposes and reassembles the full outputs.
"""

import numpy as np
import ml_dtypes

EMB = 512
HEADS = 16
WS = 8
B = 8
HW = 64
N = HW * HW
EH = EMB // HEADS          # 32
WN = HW // WS              # 8
SHIFT = WS // 2            # 4
WT = WS * WS               # 64 tokens per window
NCORES = 8
NBLK = 8                   # token blocks per core (512 tokens each)
BLKT = N // NBLK           # 512
NPAIR = 4                  # window pairs per block
PT = 2 * WT                # 128 tokens per pair
MASK_NEG = -30000.0

_bf16 = ml_dtypes.bfloat16


def _build_perm(shift):
    """perm[t] = token index n for window-ordered position t."""
    i, j, w1, w2 = np.meshgrid(
        np.arange(WN), np.arange(WN), np.arange(WS), np.arange(WS), indexing="ij"
    )
    r = (WS * i + w1 + shift) % HW
    c = (WS * j + w2 + shift) % HW
    return (r * HW + c).reshape(-1)


_PERM = _build_perm(SHIFT)
_OPERM = _build_perm(0)

# Reference splits EMB as (e H): head h lives on strided channels e*HEADS+h.
# Permute projection out-channels so head h is the contiguous block h*EH..:
_RHO = np.array([e * HEADS + h for h in range(HEADS) for e in range(EH)])


def _pair_tables(pos_emb):
    """4 pair-type tables [128, 128] bf16 of exp(T)^T with cross-window zeros.

    T[q, k] = pos_bias (+ row mask if window-row i == 7)
                       (+ col mask on the second window if its col j == 7).
    Table layout: partition = pair-k (2 windows stacked), free = pair-q.
    Cross-window blocks are exactly zero (masking + denominator correctness).
    pair p = windows (2p, 2p+1): second window col-masked iff p % 4 == 3;
    both windows row-masked iff p // 4 == WN - 1.
    """
    idx = np.array([[x, y] for x in range(WS) for y in range(WS)])
    rel = idx[None, :, :] - idx[:, None, :] + WS - 1
    bias = pos_emb[rel[:, :, 0], rel[:, :, 1]].astype(np.float64)

    m = np.zeros((WT, WT), dtype=np.float64)
    s = WS * (WS // 2)
    m[-s:, :-s] = MASK_NEG
    m[:-s, -s:] = MASK_NEG
    r = WT // WS
    col = m.reshape(r, WS, r, WS).transpose(1, 0, 3, 2).reshape(WT, WT)

    t0 = bias
    t1 = bias + m          # row-masked  (i == 7)
    t2 = bias + col        # col-masked  (j == 7)
    t3 = bias + m + col    # corner

    def pair_tab(ta, tb):
        full = np.zeros((PT, PT), dtype=np.float64)
        full[:WT, :WT] = np.exp(ta).T       # [k, q]
        full[WT:, WT:] = np.exp(tb).T
        return full.astype(_bf16)

    # pair types: (normal,normal), (normal,colmask), (rowmask,rowmask),
    # (rowmask,corner)
    return np.stack([
        pair_tab(t0, t0),
        pair_tab(t0, t2),
        pair_tab(t1, t1),
        pair_tab(t1, t3),
    ])


def _pair_type(pg):
    row = (pg // 4) == WN - 1      # window-row i == 7
    colm = (pg % 4) == 3           # second window j == 7
    return (2 if row else 0) + (1 if colm else 0)


def _build_bass():
    import concourse.mybir as mybir
    from concourse import bacc
    from concourse.tile import TileContext

    fp32 = mybir.dt.float32
    bf16 = mybir.dt.bfloat16
    AF = mybir.ActivationFunctionType
    ALU = mybir.AluOpType

    nc = bacc.Bacc()

    # ---- DRAM parameters (per-core) ----
    d_in = {}
    for name in ("cw", "sw", "scw", "shw"):
        d_in[name] = nc.declare_dram_parameter(name, [EMB, N], bf16, isOutput=False)
    for name in ("w1t", "w2t", "wsct", "wsht", "wsot", "wshot"):
        d_in[name] = nc.declare_dram_parameter(name, [EMB, EMB], bf16, isOutput=False)
    for name in ("b1r", "b2r", "bsor", "bshor"):
        d_in[name] = nc.declare_dram_parameter(name, [128, 4], fp32, isOutput=False)
    d_in["ptab"] = nc.declare_dram_parameter("ptab", [4, 128, PT], bf16,
                                             isOutput=False)
    d_in["ones32"] = nc.declare_dram_parameter("ones32", [128, 32], bf16,
                                               isOutput=False)
    yso = nc.declare_dram_parameter("yso", [EMB, N], bf16, isOutput=True)
    ysho = nc.declare_dram_parameter("ysho", [EMB, N], bf16, isOutput=True)

    with TileContext(nc) as tc:
        with (
            tc.tile_pool(name="const", bufs=1) as cpool,
            tc.tile_pool(name="xg", bufs=2) as xgpool,
            tc.tile_pool(name="cs", bufs=2) as cspool,
            tc.tile_pool(name="v", bufs=3) as vpool,
            tc.tile_pool(name="pe", bufs=2) as pepool,
            tc.tile_pool(name="pa", bufs=6) as papool,
            tc.tile_pool(name="rd", bufs=2) as rdpool,
            tc.tile_pool(name="o", bufs=2) as opool,
            tc.tile_pool(name="y", bufs=2) as ypool,
            tc.tile_pool(name="stps", bufs=2, space="PSUM") as stps,
            tc.tile_pool(name="avps", bufs=2, space="PSUM") as avps,
            tc.tile_pool(name="ddps", bufs=2, space="PSUM") as ddps,
            tc.tile_pool(name="pjps", bufs=2, space="PSUM") as pjps,
        ):
            # ---- constants into SBUF ----
            wts = {}
            for name in ("w1t", "w2t", "wsct", "wsht", "wsot", "wshot"):
                t = cpool.tile([128, 4, EMB], bf16, tag=name)
                nc.sync.dma_start(
                    t[:], d_in[name][:].rearrange("(k p) e -> p k e", p=128)
                )
                wts[name] = t
            bias_t = {}
            for name in ("b1r", "b2r", "bsor", "bshor"):
                t = cpool.tile([128, 4], fp32, tag=name)
                nc.sync.dma_start(t[:], d_in[name][:])
                bias_t[name] = t
            ptab_t = []
            for i in range(4):
                t = cpool.tile([128, PT], bf16, tag=f"ptab{i}")
                nc.sync.dma_start(t[:], d_in["ptab"][i])
                ptab_t.append(t)
            ones32 = cpool.tile([128, 32], bf16, tag="ones32")
            nc.sync.dma_start(ones32[:], d_in["ones32"][:])

            for blk in range(NBLK):
                c0 = blk * BLKT
                # ---- stage inputs [128, k-chunk, 512 tokens] ----
                xg = {}
                for tname in ("cw", "sw", "scw", "shw"):
                    t = xgpool.tile([128, 4, BLKT], bf16, tag=f"xg_{tname}")
                    nc.sync.dma_start(
                        t[:],
                        d_in[tname][:, c0:c0 + BLKT].rearrange(
                            "(k p) t -> p k t", p=128
                        ),
                    )
                    xg[tname] = t

                # ---- Q/K projections (feature-major) ----
                cs = {}
                for tname, wname, bname in (
                    ("cw", "w1t", "b1r"), ("sw", "w2t", "b2r")
                ):
                    cst = cspool.tile([128, 4, BLKT], bf16, tag=f"cs_{tname}")
                    for m in range(4):
                        ps = pjps.tile([128, BLKT], fp32, tag="pj", name="pspj")
                        for k in range(4):
                            nc.tensor.matmul(
                                ps[:],
                                lhsT=wts[wname][:, k, m * 128:(m + 1) * 128],
                                rhs=xg[tname][:, k, :],
                                start=(k == 0),
                                stop=(k == 3),
                            )
                        nc.scalar.activation(
                            cst[:, m, :], ps[:], AF.Identity,
                            bias=bias_t[bname][:, m:m + 1],
                        )
                    cs[tname] = cst
                cT, sT = cs["cw"], cs["sw"]

                # ---- V projections (token-major) into per-pair Vcomb ----
                # Vcomb[p] layout: [128 tok, 16h * (32 sc | 32 sh)]
                vcomb = []
                for p in range(4):
                    t0 = p * PT
                    vc = vpool.tile([128, HEADS, 2, EH], bf16, tag="vcomb")
                    for vi, (tname, wname) in enumerate(
                        (("scw", "wsct"), ("shw", "wsht"))
                    ):
                        ps = pjps.tile([128, EMB], fp32, tag="pj", name="psv")
                        for k in range(4):
                            nc.tensor.matmul(
                                ps[:],
                                lhsT=xg[tname][:, k, t0:t0 + PT],
                                rhs=wts[wname][:, k, :],
                                start=(k == 0),
                                stop=(k == 3),
                            )
                        nc.vector.tensor_copy(
                            vc[:, :, vi, :],
                            ps[:].rearrange("p (h e) -> p h e", h=HEADS),
                        )
                    vcomb.append(vc)

                # ---- attention per pair ----
                # O feature-major per block: [128, 4 m-chunk, 512 tok] bf16
                osc = opool.tile([128, 4, BLKT], bf16, tag="osc")
                osh = opool.tile([128, 4, BLKT], bf16, tag="osh")
                for p in range(4):
                    pg = blk * 4 + p
                    ptab = ptab_t[_pair_type(pg)]
                    t0 = p * PT
                    # S^T + exp + table, one quarter at a time. Quarter c =
                    # heads {4m+c}, all at PE row-group 32c: same-tile matmuls
                    # serialize; cross-quarter ones hit different psum banks,
                    # so no (bank, partition) write collisions.
                    pa_q = []
                    for c in range(4):
                        r = c * 32
                        st = stps.tile([128, 4, PT], fp32, tag="st")
                        for m in range(4):
                            nc.tensor.matmul(
                                st[:, m, :],
                                lhsT=sT[r:r + 32, m, t0:t0 + PT],
                                rhs=cT[r:r + 32, m, t0:t0 + PT],
                                start=True, stop=True,
                                tile_position=(r, 0),
                            )
                        pe = pepool.tile([128, 4, PT], bf16, tag="pe")
                        nc.scalar.activation(pe[:], st[:], AF.Exp)
                        pa = papool.tile([128, 4, PT], bf16, tag="pa")
                        nc.vector.tensor_tensor(
                            pa[:], pe[:],
                            ptab[:, None, :].broadcast_to([128, 4, PT]),
                            ALU.mult,
                        )
                        pa_q.append(pa)

                    # denominators of head 4m+c, replicated onto partitions
                    # 32c..32c+32 at chunk column m (matching the AV layout)
                    dd = ddps.tile([128, 4, PT], fp32, tag="dd")
                    for c in range(4):
                        r = c * 32
                        for m in range(4):
                            nc.tensor.matmul(
                                dd[r:r + 32, m, :],
                                lhsT=ones32[:, :],
                                rhs=pa_q[c][:, m, :],
                                start=True, stop=True,
                                tile_position=(0, r),
                            )
                    rd = rdpool.tile([128, 4, PT], fp32, tag="rd")
                    nc.vector.reciprocal(rd[:], dd[:])

                    # AV: head 4m+c emits [32e, 128q] at partition 32c,
                    # chunk column m -> psum is feature-major O directly
                    for vi, av_name in ((0, "avsc"), (1, "avsh")):
                        av = avps.tile([128, 4, PT], fp32, tag="av",
                                       name=av_name)
                        for c in range(4):
                            r = c * 32
                            for m in range(4):
                                nc.tensor.matmul(
                                    av[r:r + 32, m, :],
                                    lhsT=vcomb[p][:, 4 * m + c, vi, :],
                                    rhs=pa_q[c][:, m, :],
                                    start=True, stop=True,
                                    tile_position=(0, r),
                                )
                        o_t = osc if vi == 0 else osh
                        nc.vector.tensor_tensor(
                            o_t[:, :, t0:t0 + PT], av[:], rd[:], ALU.mult
                        )

                # ---- output projections ----
                for o_t, wname, bname, y_h in (
                    (osc, "wsot", "bsor", yso), (osh, "wshot", "bshor", ysho)
                ):
                    y_sb = ypool.tile([128, 4, BLKT], bf16, tag="y")
                    for mo in range(4):
                        ps = pjps.tile([128, BLKT], fp32, tag="pj", name="pso")
                        for k in range(4):
                            nc.tensor.matmul(
                                ps[:],
                                lhsT=wts[wname][:, k, mo * 128:(mo + 1) * 128],
                                rhs=o_t[:, k, :],
                                start=(k == 0),
                                stop=(k == 3),
                            )
                        nc.scalar.activation(
                            y_sb[:, mo, :], ps[:], AF.Identity,
                            bias=bias_t[bname][:, mo:mo + 1],
                        )
                    nc.sync.dma_start(
                        y_h[:, c0:c0 + BLKT].rearrange("(m p) t -> p m t", p=128),
                        y_sb[:],
                    )
    nc.compile()
    return nc


_NC_CACHE = {}
LAST_RESULT = None


def make_in_maps(content, style, scale, shift, W1, b1, W2, b2, Wsc, bsc,
                 Wsh, bsh, Wso, bso, Wsho, bsho, pos_emb):
    inv = 1.0 / np.sqrt(EMB / HEADS)
    f32 = np.float32

    w1t = (np.asarray(W1, f32)[_RHO].T * inv).astype(_bf16)
    w2t = np.asarray(W2, f32)[_RHO].T.astype(_bf16)
    wsct = np.asarray(Wsc, f32)[_RHO].T.astype(_bf16)
    wsht = np.asarray(Wsh, f32)[_RHO].T.astype(_bf16)
    # attention output channels are H-major (h*EH+e), so the output
    # projections are NOT channel-permuted.
    wsot = np.asarray(Wso, f32).T.astype(_bf16)
    wshot = np.asarray(Wsho, f32).T.astype(_bf16)
    b1r = (np.asarray(b1, f32)[_RHO] * inv).reshape(4, 128).T.copy()
    b2r = np.asarray(b2, f32)[_RHO].reshape(4, 128).T.copy()
    # V biases folded into output-projection biases (attention rows sum to 1).
    bso2 = np.asarray(Wso, f32) @ np.asarray(bsc, f32)[_RHO] + np.asarray(bso, f32)
    bsho2 = (np.asarray(Wsho, f32) @ np.asarray(bsh, f32)[_RHO]
             + np.asarray(bsho, f32))
    bsor = bso2.reshape(4, 128).T.copy()
    bshor = bsho2.reshape(4, 128).T.copy()
    ptab = _pair_tables(np.asarray(pos_emb, f32))
    ones32 = np.ones((128, 32), dtype=_bf16)

    common = dict(
        w1t=w1t, w2t=w2t, wsct=wsct, wsht=wsht, wsot=wsot, wshot=wshot,
        b1r=b1r, b2r=b2r, bsor=bsor, bshor=bshor, ptab=ptab, ones32=ones32,
    )
    in_maps = []
    for b in range(NCORES):
        m = dict(common)
        for name, full in (("cw", content), ("sw", style),
                           ("scw", scale), ("shw", shift)):
            x = np.asarray(full[b], f32)[_PERM]           # [N, EMB] window order
            m[name] = np.ascontiguousarray(x.T).astype(_bf16)
        in_maps.append(m)
    return in_maps


def kernel(**inputs):
    global LAST_RESULT
    from concourse.bass_utils import run_bass_kernel_spmd

    in_maps = make_in_maps(**inputs)

    if "nc" not in _NC_CACHE:
        _NC_CACHE["nc"] = _build_bass()
    res = run_bass_kernel_spmd(_NC_CACHE["nc"], in_maps, list(range(NCORES)))
    LAST_RESULT = res

    out_sc = np.empty((B, N, EMB), np.float32)
    out_sh = np.empty((B, N, EMB), np.float32)
    for b in range(NCORES):
        out_sc[b][_OPERM] = res.results[b]["yso"].T.astype(np.float32)
        out_sh[b][_OPERM] = res.results[b]["ysho"].T.astype(np.float32)
    return out_sc, out_sh


# revision 47
# speedup vs baseline: 1.5763x; 1.3469x over previous
"""Trainium2 Bass kernel for nn_DecoderCrossMSA (Swin-style shifted-window
cross-attention).

Strategy: data-parallel over batch (8 batches -> 8 cores). Host prepares, per
core, feature-major window-ordered activations (token axis permuted so every
8x8 shifted window is a contiguous 64-token run; roll folded into the
permutation). Device, per 512-token block (4 window-pairs):
  - Q/K projections feature-major, V projections token-major into a per-pair
    combined tile (sc|sh interleaved per head), all bf16 (Q pre-scaled by
    1/sqrt(32); V biases folded into the output-projection biases),
  - per pair (2 windows, 128 tokens): S^T = K^T.T @ Q^T per head over the
    full 128x128 pair block (cross-window entries junk), exp on the scalar
    engine, multiplied by a static table exp(pos bias + shift masks) with
    zeros on cross-window blocks (masking = exact zeros, head-broadcast via
    stride-0 APs),
  - denominators replicated onto 32-partition groups via ones-lhsT matmuls,
    reciprocal on DVE, one broadcast tensor_tensor per (pair, out) that
    normalizes AND scatters attention output feature-major,
  - AV matmuls with V stationary emit [32e, 128q] per head at the partition
    offset that makes psum columns land feature-major directly,
  - output projections in bf16, bf16 out (host converts to fp32).
Host inverse-permutes/transposes and reassembles the full outputs.
"""

import numpy as np
import ml_dtypes

EMB = 512
HEADS = 16
WS = 8
B = 8
HW = 64
N = HW * HW
EH = EMB // HEADS          # 32
WN = HW // WS              # 8
SHIFT = WS // 2            # 4
WT = WS * WS               # 64 tokens per window
NCORES = 8
NBLK = 8                   # token blocks per core (512 tokens each)
BLKT = N // NBLK           # 512
NPAIR = 4                  # window pairs per block
PT = 2 * WT                # 128 tokens per pair
MASK_NEG = -30000.0

_bf16 = ml_dtypes.bfloat16


def _build_perm(shift):
    """perm[t] = token index n for window-ordered position t."""
    i, j, w1, w2 = np.meshgrid(
        np.arange(WN), np.arange(WN), np.arange(WS), np.arange(WS), indexing="ij"
    )
    r = (WS * i + w1 + shift) % HW
    c = (WS * j + w2 + shift) % HW
    return (r * HW + c).reshape(-1)


_PERM = _build_perm(SHIFT)
_OPERM = _build_perm(0)

# Reference splits EMB as (e H): head h lives on strided channels e*HEADS+h.
# Permute projection out-channels so head h is the contiguous block h*EH..:
_RHO = np.array([e * HEADS + h for h in range(HEADS) for e in range(EH)])


def _pair_tables(pos_emb):
    """4 pair-type tables [128, 128] bf16 of exp(T)^T with cross-window zeros.

    T[q, k] = pos_bias (+ row mask if window-row i == 7)
                       (+ col mask on the second window if its col j == 7).
    Table layout: partition = pair-k (2 windows stacked), free = pair-q.
    Cross-window blocks are exactly zero (masking + denominator correctness).
    pair p = windows (2p, 2p+1): second window col-masked iff p % 4 == 3;
    both windows row-masked iff p // 4 == WN - 1.
    """
    idx = np.array([[x, y] for x in range(WS) for y in range(WS)])
    rel = idx[None, :, :] - idx[:, None, :] + WS - 1
    bias = pos_emb[rel[:, :, 0], rel[:, :, 1]].astype(np.float64)

    m = np.zeros((WT, WT), dtype=np.float64)
    s = WS * (WS // 2)
    m[-s:, :-s] = MASK_NEG
    m[:-s, -s:] = MASK_NEG
    r = WT // WS
    col = m.reshape(r, WS, r, WS).transpose(1, 0, 3, 2).reshape(WT, WT)

    t0 = bias
    t1 = bias + m          # row-masked  (i == 7)
    t2 = bias + col        # col-masked  (j == 7)
    t3 = bias + m + col    # corner

    def pair_tab(ta, tb):
        full = np.zeros((PT, PT), dtype=np.float64)
        full[:WT, :WT] = np.exp(ta).T       # [k, q]
        full[WT:, WT:] = np.exp(tb).T
        return full.astype(_bf16)

    # pair types: (normal,normal), (normal,colmask), (rowmask,rowmask),
    # (rowmask,corner)
    return np.stack([
        pair_tab(t0, t0),
        pair_tab(t0, t2),
        pair_tab(t1, t1),
        pair_tab(t1, t3),
    ])


def _pair_type(pg):
    row = (pg // 4) == WN - 1      # window-row i == 7
    colm = (pg % 4) == 3           # second window j == 7
    return (2 if row else 0) + (1 if colm else 0)


def _build_bass(debug=False):
    import concourse.mybir as mybir
    from concourse import bacc
    from concourse.tile import TileContext

    fp32 = mybir.dt.float32
    bf16 = mybir.dt.bfloat16
    AF = mybir.ActivationFunctionType
    ALU = mybir.AluOpType

    nc = bacc.Bacc()

    # ---- DRAM parameters (per-core) ----
    d_in = {}
    for name in ("cw", "sw", "scw", "shw"):
        d_in[name] = nc.declare_dram_parameter(name, [EMB, N], bf16, isOutput=False)
    for name in ("w1t", "w2t", "wsct", "wsht", "wsot", "wshot"):
        d_in[name] = nc.declare_dram_parameter(name, [EMB, EMB], bf16, isOutput=False)
    for name in ("b1r", "b2r", "bsor", "bshor"):
        d_in[name] = nc.declare_dram_parameter(name, [128, 4], fp32, isOutput=False)
    d_in["ptab"] = nc.declare_dram_parameter("ptab", [4, 128, PT], bf16,
                                             isOutput=False)
    d_in["ones1"] = nc.declare_dram_parameter("ones1", [128, 1], bf16,
                                              isOutput=False)
    yso = nc.declare_dram_parameter("yso", [EMB, N], bf16, isOutput=True)
    ysho = nc.declare_dram_parameter("ysho", [EMB, N], bf16, isOutput=True)
    dbg = {}
    if debug:
        for name, shape in (
            ("dbg_ot", [128, EMB]), ("dbg_ofm", [4, 128, BLKT]),
            ("dbg_pa", [128, 4, PT]), ("dbg_rd", [128, 16]),
        ):
            dbg[name] = nc.declare_dram_parameter(name, shape, fp32,
                                                  isOutput=True)

    with TileContext(nc) as tc:
        with (
            tc.tile_pool(name="const", bufs=1) as cpool,
            tc.tile_pool(name="xg", bufs=3) as xgpool,
            tc.tile_pool(name="cs", bufs=2) as cspool,
            tc.tile_pool(name="v", bufs=9) as vpool,
            tc.tile_pool(name="pe", bufs=4) as pepool,
            tc.tile_pool(name="pa", bufs=8) as papool,
            tc.tile_pool(name="rd", bufs=2) as rdpool,
            tc.tile_pool(name="ot", bufs=3) as otpool,
            tc.tile_pool(name="o", bufs=2) as opool,
            tc.tile_pool(name="y", bufs=3) as ypool,
            tc.tile_pool(name="od", bufs=2, space="DRAM") as odpool,
            tc.tile_pool(name="stps", bufs=3, space="PSUM") as stps,
            tc.tile_pool(name="avps", bufs=2, space="PSUM") as avps,
            tc.tile_pool(name="ddps", bufs=1, space="PSUM") as ddps,
            tc.tile_pool(name="pjps", bufs=2, space="PSUM") as pjps,
        ):
            # ---- constants into SBUF ----
            wts = {}
            for name in ("w1t", "w2t", "wsct", "wsht", "wsot", "wshot"):
                t = cpool.tile([128, 4, EMB], bf16, tag=name)
                nc.sync.dma_start(
                    t[:], d_in[name][:].rearrange("(k p) e -> p k e", p=128)
                )
                wts[name] = t
            bias_t = {}
            for name in ("b1r", "b2r", "bsor", "bshor"):
                t = cpool.tile([128, 4], fp32, tag=name)
                nc.sync.dma_start(t[:], d_in[name][:])
                bias_t[name] = t
            ptab_t = []
            for i in range(4):
                t = cpool.tile([128, PT], bf16, tag=f"ptab{i}")
                nc.sync.dma_start(t[:], d_in["ptab"][i])
                ptab_t.append(t)
            ones1 = cpool.tile([128, 1], bf16, tag="ones1")
            nc.sync.dma_start(ones1[:], d_in["ones1"][:])

            # ---- stage all inputs up front [128, k-chunk, 512 tokens] ----
            xg_all = []
            for blk in range(NBLK):
                c0 = blk * BLKT
                xg = {}
                for tname in ("cw", "sw", "scw", "shw"):
                    t = xgpool.tile([128, 4, BLKT], bf16, tag=f"xg_{tname}")
                    nc.sync.dma_start(
                        t[:],
                        d_in[tname][:, c0:c0 + BLKT].rearrange(
                            "(k p) t -> p k t", p=128
                        ),
                    )
                    xg[tname] = t
                xg_all.append(xg)

            # ---- software-pipelined block emission ----
            # While attending block b, interleave (between quarter chains):
            # Q/K/V projection jobs for block b+1 and output-projection jobs
            # for block b-1. Keeps the in-order PE queue fed during the
            # S -> exp -> table -> AV dependency chains.
            from collections import deque

            cs_blk = {}     # b -> {"cw": tile, "sw": tile}
            vcomb_blk = {}  # b -> [4 vcomb tiles]
            ofm_blk = {}    # b -> [8 ofm tiles]

            def qkv_jobs(b):
                """Prep jobs for block b: 8 QK groups + 8 V groups."""
                xg = xg_all[b]
                cs = {}
                for tname in ("cw", "sw"):
                    cs[tname] = cspool.tile([128, 4, BLKT], bf16,
                                            tag=f"cs_{tname}", name="cst")
                cs_blk[b] = cs
                vcomb_blk[b] = [
                    vpool.tile([128, 2, HEADS, EH], bf16, tag="vcomb",
                               name="vc")
                    for _ in range(4)
                ]
                jobs = []

                def qk_group(tname, wname, bname, m):
                    def run():
                        ps = pjps.tile([128, BLKT], fp32, tag="pj", name="pspj")
                        for k in range(4):
                            nc.tensor.matmul(
                                ps[:],
                                lhsT=wts[wname][:, k, m * 128:(m + 1) * 128],
                                rhs=xg[tname][:, k, :],
                                start=(k == 0),
                                stop=(k == 3),
                            )
                        # K bias dropped: constant over k, cancels in softmax
                        if bname is not None:
                            nc.scalar.activation(
                                cs_blk[b][tname][:, m, :], ps[:], AF.Identity,
                                bias=bias_t[bname][:, m:m + 1],
                            )
                        else:
                            nc.vector.tensor_copy(
                                cs_blk[b][tname][:, m, :], ps[:]
                            )
                    return run

                def v_group(p, vi, tname, wname):
                    def run():
                        t0 = p * PT
                        ps = pjps.tile([128, EMB], fp32, tag="pj", name="psv")
                        for k in range(4):
                            nc.tensor.matmul(
                                ps[:],
                                lhsT=xg[tname][:, k, t0:t0 + PT],
                                rhs=wts[wname][:, k, :],
                                start=(k == 0),
                                stop=(k == 3),
                            )
                        eng = nc.scalar if vi == 0 else nc.vector
                        if eng is nc.scalar:
                            nc.scalar.activation(
                                vcomb_blk[b][p][:, vi, :, :].rearrange(
                                    "p h e -> p (h e)"),
                                ps[:], AF.Copy,
                            )
                        else:
                            nc.vector.tensor_copy(
                                vcomb_blk[b][p][:, vi, :, :],
                                ps[:].rearrange("p (h e) -> p h e", h=HEADS),
                            )
                    return run

                for m in range(4):
                    jobs.append(qk_group("cw", "w1t", "b1r", m))
                    jobs.append(qk_group("sw", "w2t", None, m))
                for p in range(4):
                    jobs.append(v_group(p, 0, "scw", "wsct"))
                    jobs.append(v_group(p, 1, "shw", "wsht"))
                return jobs

            def out_jobs(b):
                """Finish jobs for block b: 8 output-projection groups."""
                c0b = b * BLKT
                ysb = {}
                jobs = []

                def out_group(vi, wname, bname, y_h, mo):
                    def run():
                        if mo == 0:
                            ysb[vi] = ypool.tile([128, 4, BLKT], bf16, tag="y",
                                                 name="ysb")
                        ps = pjps.tile([128, BLKT], fp32, tag="pj", name="pso")
                        for k in range(4):
                            nc.tensor.matmul(
                                ps[:],
                                lhsT=wts[wname][:, k, mo * 128:(mo + 1) * 128],
                                rhs=ofm_blk[b][vi * 4 + k][:],
                                start=(k == 0),
                                stop=(k == 3),
                            )
                        nc.scalar.activation(
                            ysb[vi][:, mo, :], ps[:], AF.Identity,
                            bias=bias_t[bname][:, mo:mo + 1],
                        )
                        if mo == 3:
                            nc.scalar.dma_start(
                                y_h[:, c0b:c0b + BLKT].rearrange(
                                    "(m p) t -> p m t", p=128
                                ),
                                ysb[vi][:],
                            )
                    return run

                for vi, (wname, bname, y_h) in enumerate(
                    (("wsot", "bsor", yso), ("wshot", "bshor", ysho))
                ):
                    for mo in range(4):
                        jobs.append(out_group(vi, wname, bname, y_h, mo))
                return jobs

            def emit_xbar(b, od_t):
                ofm_blk[b] = []
                for vi in range(2):
                    for k in range(4):
                        t = opool.tile([128, BLKT], bf16, tag=f"ofm{vi}_{k}",
                                       name="ofm")
                        nc.sync.dma_start(
                            t[:],
                            od_t[vi][:, k * 128:(k + 1) * 128],
                            transpose=True,
                        )
                        ofm_blk[b].append(t)

            # prologue: QKV of block 0 emitted plainly
            for job in qkv_jobs(0):
                job()

            for blk in range(NBLK):
                c0 = blk * BLKT
                cT, sT = cs_blk[blk]["cw"], cs_blk[blk]["sw"]
                vcomb = vcomb_blk[blk]
                # DRAM staging for this block's O^T -> O round-trip, [tok, e]
                od_t = [odpool.tile([BLKT, EMB], bf16, tag=f"od{vi}",
                                    name="odt") for vi in range(2)]

                filler = deque()
                if blk + 1 < NBLK:
                    filler.extend(qkv_jobs(blk + 1))
                if blk >= 1:
                    filler.extend(out_jobs(blk - 1))

                # ---- attention per pair ----
                for p in range(4):
                    pg = blk * 4 + p
                    ptab = ptab_t[_pair_type(pg)]
                    t0 = p * PT
                    # S^T + exp + table, one quarter at a time. Quarter c =
                    # heads {4m+c}, all at PE row-group 32c: same-tile matmuls
                    # serialize; cross-quarter ones hit different psum banks,
                    # so no (bank, partition) write collisions.
                    pa_q = []
                    dd = ddps.tile([128, 16], fp32, tag="dd")
                    for c in range(4):
                        r = c * 32
                        st = stps.tile([128, 4, PT], fp32, tag="st")
                        for m in range(4):
                            nc.tensor.matmul(
                                st[:, m, :],
                                lhsT=sT[r:r + 32, m, t0:t0 + PT],
                                rhs=cT[r:r + 32, m, t0:t0 + PT],
                                start=True, stop=True,
                                tile_position=(r, 0),
                            )
                        pe = pepool.tile([128, 4, PT], bf16, tag="pe")
                        nc.scalar.activation(pe[:], st[:], AF.Exp)
                        pa = papool.tile([128, 4, PT], bf16, tag="pa")
                        nc.vector.tensor_tensor(
                            pa[:], pe[:],
                            ptab[:, None, :].broadcast_to([128, 4, PT]),
                            ALU.mult,
                        )
                        pa_q.append(pa)
                        # denominators d[q, h] for head h = 4m+c: 1-free
                        # matmuls with the pa tile stationary, interleaved so
                        # dd completes as soon as the last quarter lands
                        for m in range(4):
                            h = 4 * m + c
                            nc.tensor.matmul(
                                dd[:, h:h + 1],
                                lhsT=pa[:, m, :],
                                rhs=ones1[:, :],
                                start=True, stop=True,
                            )
                        # fill the PE queue with independent projection work
                        # while this quarter's exp/table chain completes
                        if filler:
                            filler.popleft()()
                        if len(filler) > 12 and filler:
                            filler.popleft()()
                    rd1 = rdpool.tile([128, 16], fp32, tag="rd1")
                    nc.vector.reciprocal(rd1[:], dd[:])
                    rdb = rdpool.tile([128, 16], bf16, tag="rdb")
                    nc.vector.tensor_copy(rdb[:], rd1[:])
                    if debug and blk == 0 and p == 0:
                        nc.gpsimd.dma_start(dbg["dbg_rd"][:], rd1[:])
                        nc.gpsimd.dma_start(dbg["dbg_pa"][:], pa_q[0][:])

                    # AV (transposed): head h = 4m+c emits O^T[q, h*32:+32];
                    # av tiles are O^T pair-rows, channel order h-major.
                    # Normalization happens on evacuation: O^T = av * rd[q, h]
                    # (free-dim broadcast of rd over the 32 channels).
                    for vi in range(2):
                        av = avps.tile([128, HEADS, EH], fp32, tag="av",
                                       name=f"av{vi}")
                        for h in range(HEADS):
                            c, m = h % 4, h // 4
                            nc.tensor.matmul(
                                av[:, h, :],
                                lhsT=pa_q[c][:, m, :],
                                rhs=vcomb[p][:, vi, h, :],
                                start=True, stop=True,
                            )
                        ot = otpool.tile([128, HEADS, EH], bf16, tag=f"ot{vi}")
                        nc.vector.tensor_tensor(
                            ot[:], av[:],
                            rdb[:, :, None].broadcast_to([128, HEADS, EH]),
                            ALU.mult,
                        )
                        nc.gpsimd.dma_start(
                            od_t[vi][t0:t0 + PT, :],
                            ot[:].rearrange("p h e -> p (h e)"),
                        )
                        if debug and blk == 0 and p == 0 and vi == 0:
                            nc.gpsimd.dma_start(
                                dbg["dbg_ot"][:],
                                ot[:].rearrange("p h e -> p (h e)"),
                            )
                    # drain a filler job after each pair's AV as well
                    if filler:
                        filler.popleft()()

                # all od writes for this block are issued; start transposes
                emit_xbar(blk, od_t)
                if debug and blk == 0:
                    for k in range(4):
                        nc.gpsimd.dma_start(
                            dbg["dbg_ofm"][k], ofm_blk[0][k][:]
                        )
                # drain any remaining filler jobs
                while filler:
                    filler.popleft()()

            # epilogue: output projections of the last block
            for job in out_jobs(NBLK - 1):
                job()
    nc.compile()
    return nc


_NC_CACHE = {}
LAST_RESULT = None


def make_in_maps(content, style, scale, shift, W1, b1, W2, b2, Wsc, bsc,
                 Wsh, bsh, Wso, bso, Wsho, bsho, pos_emb):
    inv = 1.0 / np.sqrt(EMB / HEADS)
    f32 = np.float32

    w1t = (np.asarray(W1, f32)[_RHO].T * inv).astype(_bf16)
    w2t = np.asarray(W2, f32)[_RHO].T.astype(_bf16)
    wsct = np.asarray(Wsc, f32)[_RHO].T.astype(_bf16)
    wsht = np.asarray(Wsh, f32)[_RHO].T.astype(_bf16)
    # attention output channels are H-major (h*EH+e), so the output
    # projections are NOT channel-permuted.
    wsot = np.asarray(Wso, f32).T.astype(_bf16)
    wshot = np.asarray(Wsho, f32).T.astype(_bf16)
    b1r = (np.asarray(b1, f32)[_RHO] * inv).reshape(4, 128).T.copy()
    b2r = np.asarray(b2, f32)[_RHO].reshape(4, 128).T.copy()
    # V biases folded into output-projection biases (attention rows sum to 1).
    bso2 = np.asarray(Wso, f32) @ np.asarray(bsc, f32)[_RHO] + np.asarray(bso, f32)
    bsho2 = (np.asarray(Wsho, f32) @ np.asarray(bsh, f32)[_RHO]
             + np.asarray(bsho, f32))
    bsor = bso2.reshape(4, 128).T.copy()
    bshor = bsho2.reshape(4, 128).T.copy()
    ptab = _pair_tables(np.asarray(pos_emb, f32))
    ones1 = np.ones((128, 1), dtype=_bf16)

    common = dict(
        w1t=w1t, w2t=w2t, wsct=wsct, wsht=wsht, wsot=wsot, wshot=wshot,
        b1r=b1r, b2r=b2r, bsor=bsor, bshor=bshor, ptab=ptab, ones1=ones1,
    )
    in_maps = []
    for b in range(NCORES):
        m = dict(common)
        for name, full in (("cw", content), ("sw", style),
                           ("scw", scale), ("shw", shift)):
            x = np.asarray(full[b], f32)[_PERM]           # [N, EMB] window order
            m[name] = np.ascontiguousarray(x.T).astype(_bf16)
        in_maps.append(m)
    return in_maps


def kernel(**inputs):
    global LAST_RESULT
    from concourse.bass_utils import run_bass_kernel_spmd

    in_maps = make_in_maps(**inputs)

    if "nc" not in _NC_CACHE:
        _NC_CACHE["nc"] = _build_bass()
    res = run_bass_kernel_spmd(_NC_CACHE["nc"], in_maps, list(range(NCORES)))
    LAST_RESULT = res

    out_sc = np.empty((B, N, EMB), np.float32)
    out_sh = np.empty((B, N, EMB), np.float32)
    for b in range(NCORES):
        out_sc[b][_OPERM] = res.results[b]["yso"].T.astype(np.float32)
        out_sh[b][_OPERM] = res.results[b]["ysho"].T.astype(np.float32)
    return out_sc, out_sh
